# revision 21
# baseline (speedup 1.0000x reference)
"""Bass/Trainium2 kernel for the (dead-attention) GAT reference.

Effective math (see reference):
    h1  = x @ W1f                 W1f = W1.transpose(1,0,2).reshape(256,128)
    hp1 = elu(adj @ h1)
    h2  = hp1 @ W2f               W2f = W2.transpose(1,0,2).reshape(128,128)
    hp2 = elu(adj @ h2)
    y   = elu(hp2 @ Wout + bout)
    out = log_softmax(y, axis=1)

Distribution: adj is sharded row-wise across 8 cores (2048 rows each),
uploaded pre-transposed + fp16, with the CONTRACTION rows rotated per
core so each core's own nodes come first.  h1 is computed REPLICATED on
every core (x is tiny), so layer 1 needs no collective.  h2 is
exchanged with one fp16 AllGather in feature-major layout; thanks to
the rotation each core starts layer 2 on its own h2 shard (local, no
collective wait) while the AllGather flies, then pulls the other 7
blocks with partition-id-indexed dynamic DMAs and transposes them back
to node-major lhsT tiles on the PE.  Each core streams its 67 MB adj
shard from HBM through the PE array twice:
    hpT[128 feat, 2048 rows] = sum_k h[kblk 128 rows].T-stationary @ adjT[kblk]
fp32 accumulation in PSUM; fp16 on the streamed matmuls (max elementwise
rel err vs the fp32 reference ~6e-4).
"""

import sys

import numpy as np

sys.path.insert(0, "/opt/trn_rl_repo")

N = 16384  # nodes
F = 256  # input features
D = 128  # hidden width (nheads*nhid)
C = 32  # classes
NCORES = 8
S = N // NCORES  # rows per core

_nc_cache = {}


def build_gat_nc(n_total=N, ncores=NCORES, enable_asserts=False, adj_bufs=7, kg=4):
    """Build the SPMD Bass program (one program, runs on all cores)."""
    from concourse import bacc, bass, masks, mybir, tile

    s = n_total // ncores  # shard rows per core
    kb = n_total // 128  # contraction blocks for the big matmul
    kb8 = kb // 8  # x chunk groups
    rc = s // 128  # 128-row chunks in this core's shard
    f32 = mybir.dt.float32
    f16 = mybir.dt.float16
    AF = mybir.ActivationFunctionType
    OP = mybir.AluOpType
    # n-chunks of the big-matmul output (<=512 fp32 per PSUM bank)
    nw = [min(512, s - i) for i in range(0, s, 512)]
    no = [i for i in range(0, s, 512)]

    nc = bacc.Bacc(
        "TRN2",
        target_bir_lowering=False,
        debug=False,
        enable_asserts=enable_asserts,
        num_devices=ncores,
    )

    adjt = nc.dram_tensor("adjt", [n_total, s], f16, kind="ExternalInput")
    xc = nc.dram_tensor("xc", [kb8 * 128, 8 * F], f16, kind="ExternalInput")
    w1 = nc.dram_tensor("w1", [F, D], f16, kind="ExternalInput")
    w2 = nc.dram_tensor("w2", [D, D], f16, kind="ExternalInput")
    wout = nc.dram_tensor("wout", [D, C], f32, kind="ExternalInput")
    bb = nc.dram_tensor("bb", [128, C], f32, kind="ExternalInput")
    # hoff[0, g] = ((rank + 1 + g) % ncores) * 128: gather-block row offsets
    hoff = nc.dram_tensor("hoff", [1, 8], mybir.dt.uint32, kind="ExternalInput")
    out = nc.dram_tensor("out", [s, C], f32, kind="ExternalOutput")

    rg = [list(range(ncores))]

    with tile.TileContext(nc) as tc:
        with (
            tc.tile_pool(name="dram", bufs=1, space="DRAM") as dram,
            tc.tile_pool(name="const", bufs=1) as const,
            tc.tile_pool(name="hfull", bufs=1) as hpool,
            tc.tile_pool(name="adjs", bufs=adj_bufs) as apool,
            tc.tile_pool(name="hblkp", bufs=2) as hblkp,
            tc.tile_pool(name="xcp", bufs=1) as xcpool,
            tc.tile_pool(name="xe", bufs=2) as xepool,
            tc.tile_pool(name="hsb", bufs=2) as hsbpool,
            tc.tile_pool(name="tmp", bufs=1) as tmp,
            tc.tile_pool(name="outp", bufs=2) as outp,
            tc.tile_pool(name="stat", bufs=1) as stat,
            tc.tile_pool(name="psb", bufs=4, space="PSUM") as psb,
            tc.tile_pool(name="pss", bufs=2, space="PSUM") as pss,
            tc.tile_pool(name="psy", bufs=2, space="PSUM") as psy,
        ):
            # two HWDGE rings (sync/scalar) alternate the big adj stream;
            # constants + tiny stores go to the SWDGE path (gpsimd)
            ringA, ringB, ringC = nc.sync, nc.scalar, nc.gpsimd

            # --- replicated constants (SWDGE so rings start streaming) ---
            w1s = const.tile([128, 2, D], f16, tag="w1s")
            ringC.dma_start(w1s[:], w1.ap().rearrange("(a p) m -> p a m", p=128))
            w2s = const.tile([128, D], f16, tag="w2s")
            ringC.dma_start(w2s[:], w2.ap())
            wouts = const.tile([128, C], f32, tag="wouts")
            ringC.dma_start(wouts[:], wout.ap())
            bbs = const.tile([128, C], f32, tag="bbs")
            ringC.dma_start(bbs[:], bb.ap())
            hoffs = const.tile([1, 8], mybir.dt.uint32, tag="hoffs")
            ringC.dma_start(hoffs[:], hoff.ap())
            ident = const.tile([128, 128], f16, tag="ident")
            masks.make_identity(nc, ident[:])

            # --- DRAM bounce buffers for the collective (feature-major) ---
            h2b = dram.tile([128, s], f16, tag="h2b")
            h2f = dram.tile([128 * ncores, s], f16, tag="h2f", addr_space="Shared")

            def big_layer(hs):
                # hpT[128 feat, s rows] += h[kblk].T-stationary @ adjT[kblk]
                ps = [
                    psb.tile([128, w], f32, tag="big", name=f"pbig{i}")
                    for i, w in enumerate(nw)
                ]
                ar = adjt.ap().rearrange("(g j p) m -> g p j m", j=kg, p=128)
                for g in range(kb // kg):
                    at = apool.tile([128, kg, s], f16, tag="adj")
                    (ringA if g % 2 == 0 else ringB).dma_start(at[:], ar[g])
                    for j in range(kg):
                        k = g * kg + j
                        for n, (o, w) in enumerate(zip(no, nw)):
                            nc.tensor.matmul(
                                ps[n][:],
                                hs[:, k, :],
                                at[:, j, o : o + w],
                                start=(k == 0),
                                stop=(k == kb - 1),
                            )
                return ps

            def elu_chunks(ps, dst):
                # dst[:, s] = elu(ps chunks), fp32
                for n, (o, w) in enumerate(zip(no, nw)):
                    neg = tmp.tile([128, 512], f32, tag="neg", name=f"neg{n}")
                    nc.vector.tensor_scalar_min(neg[:, :w], ps[n][:], 0.0)
                    ex = tmp.tile([128, 512], f32, tag="ex", name=f"ex{n}")
                    nc.scalar.activation(ex[:, :w], neg[:, :w], AF.Exp)
                    pm1 = tmp.tile([128, 512], f32, tag="pm1", name=f"pm1{n}")
                    nc.vector.tensor_scalar(
                        pm1[:, :w], ps[n][:], 0.0, -1.0, op0=OP.max, op1=OP.add
                    )
                    nc.vector.tensor_add(dst[:, o : o + w], ex[:, :w], pm1[:, :w])

            # ---- layer 1: h1 replicated (no collective) ----
            # xc group g holds 8 chunk-lhsTs contiguous per partition:
            # xc[g*128+p, ((j*2+a)*128)+m] = xrot.T[a*128+p, (g*8+j)*128+m]
            hs1 = hpool.tile([128, kb, D], f16, tag="hfull")
            xr = xc.ap().rearrange("(g p) q -> g p q", p=128)
            xg = None
            for k in range(kb):
                g, j = divmod(k, 8)
                if j == 0:
                    xg = xcpool.tile([128, 8, 2, 128], f16, tag="xg")
                    (ringA if g % 2 == 0 else ringB).dma_start(
                        xg.rearrange("p j a m -> p (j a m)"), xr[g]
                    )
                ph = pss.tile([128, D], f32, tag="pss", name=f"ph1_{k}")
                nc.tensor.matmul(
                    ph[:], xg[:, j, 0, :], w1s[:, 0, :], start=True, stop=False
                )
                nc.tensor.matmul(
                    ph[:], xg[:, j, 1, :], w1s[:, 1, :], start=False, stop=True
                )
                nc.vector.tensor_copy(hs1[:, k, :], ph[:])
            ps1 = big_layer(hs1)
            x2t = xepool.tile([128, s], f32, tag="xe")
            elu_chunks(ps1, x2t)

            # ---- layer 2 ----
            # own h2 shard (feature-major), start collective, and immediately
            # transpose the local shard into the first rc lhsT chunks
            x2h = xepool.tile([128, s], f16, tag="xeh")
            nc.vector.tensor_copy(x2h[:], x2t[:])
            h2sT = xepool.tile([128, s], f16, tag="h2sT")
            for c in range(rc):
                cs = slice(c * 128, (c + 1) * 128)
                ph2 = pss.tile([128, D], f32, tag="pss", name=f"ph2_{c}")
                # feat-major h2 chunk: W2f.T-stationary @ x2[feat, nodes]
                nc.tensor.matmul(ph2[:], w2s[:], x2h[:, cs], start=True, stop=True)
                nc.vector.tensor_copy(h2sT[:, cs], ph2[:])
            ringC.dma_start(h2b[:], h2sT[:])
            nc.gpsimd.collective_compute(
                "AllGather",
                OP.bypass,
                ins=[h2b.opt()],
                outs=[h2f.opt()],
                replica_groups=rg,
            )
            hs2 = hpool.tile([128, kb, D], f16, tag="hfull")
            for k in range(rc):  # own block: no collective wait
                pt = pss.tile([128, D], f16, tag="pss", name=f"ptl_{k}")
                nc.tensor.transpose(
                    pt[:], h2sT[:, k * 128 : (k + 1) * 128], ident[:]
                )
                nc.vector.tensor_copy(hs2[:, k, :], pt[:])
            # other ranks' blocks: dynamic row offset ((me+1+g) % ncores)*128
            # NOTE: keep these off the sync ring — SP-engine DMAs touching
            # collective-output buffers can hang (test_sync_dma_collective_hang)
            for g in range(ncores - 1):
                with ringB.register(f"hoffr{g}") as hreg:
                    ringB.reg_load(hreg, hoffs[0:1, g : g + 1])
                    off = ringB.snap(hreg, min_val=0, max_val=(ncores - 1) * 128)
                hb = hblkp.tile([128, s], f16, tag="hblk", name=f"hblk{g}")
                ringB.dma_start(hb[:], h2f[bass.ds(off, 128), :])
                for jj in range(rc):
                    k = rc * (1 + g) + jj
                    pt = pss.tile([128, D], f16, tag="pss", name=f"pt_{k}")
                    nc.tensor.transpose(
                        pt[:], hb[:, jj * 128 : (jj + 1) * 128], ident[:]
                    )
                    nc.vector.tensor_copy(hs2[:, k, :], pt[:])
            ps2 = big_layer(hs2)
            x3t = xepool.tile([128, s], f32, tag="xe")
            elu_chunks(ps2, x3t)

            # ---- output layer + log_softmax ----
            outr = out.ap().rearrange("(c p) m -> c p m", p=128)
            zbig = outp.tile([128, rc, C], f32, tag="zbig", bufs=1)
            for c in range(rc):
                py = psy.tile([128, C], f32, tag="psy")
                cs = slice(c * 128, (c + 1) * 128)
                nc.tensor.matmul(py[:], x3t[:, cs], wouts[:], start=True, stop=True)
                nc.vector.tensor_add(zbig[:, c, :], py[:], bbs[:])
            # batched elu over [128, rc*C]
            zf = zbig.rearrange("p c m -> p (c m)")
            negb = tmp.tile([128, rc * C], f32, tag="neg", name="negb")
            nc.vector.tensor_scalar_min(negb[:], zf, 0.0)
            eb = tmp.tile([128, rc * C], f32, tag="ex", name="eb")
            nc.scalar.activation(eb[:], negb[:], AF.Exp)
            pmb = tmp.tile([128, rc * C], f32, tag="pm1", name="pmb")
            nc.vector.tensor_scalar(pmb[:], zf, 0.0, -1.0, op0=OP.max, op1=OP.add)
            zzb = outp.tile([128, rc, C], f32, tag="zzb", bufs=1)
            nc.vector.tensor_add(
                zzb.rearrange("p c m -> p (c m)"), eb[:], pmb[:]
            )
            # batched row-max (negated), then per-chunk exp/lse/final
            negm = stat.tile([128, rc], f32, tag="negm")
            nc.vector.tensor_reduce(
                negm[:], zzb[:], axis=mybir.AxisListType.X, op=OP.max, negate=True
            )
            ssum = stat.tile([128, rc], f32, tag="ssum")
            es = tmp.tile([128, rc * C], f32, tag="neg", name="es")
            esv = es.rearrange("p (c m) -> p c m", m=C)
            for c in range(rc):
                nc.scalar.activation(
                    esv[:, c, :],
                    zzb[:, c, :],
                    AF.Exp,
                    bias=negm[:, c : c + 1],
                    accum_out=ssum[:, c : c + 1],
                )
            lse = stat.tile([128, rc], f32, tag="lse")
            nc.scalar.activation(lse[:], ssum[:], AF.Ln)
            for c in range(rc):
                osb = outp.tile([128, C], f32, tag="osb")
                nc.vector.tensor_scalar(
                    osb[:],
                    zzb[:, c, :],
                    negm[:, c : c + 1],
                    lse[:, c : c + 1],
                    op0=OP.add,
                    op1=OP.subtract,
                )
                ringC.dma_start(outr[c], osb[:])

    nc.compile()
    return nc


def make_in_maps(x, adj, W1, W2, Wout, bout, ncores=NCORES):
    n_total = adj.shape[0]
    s = n_total // ncores
    kb = n_total // 128
    kb8 = kb // 8
    f, d = W1.shape[1], W1.shape[0] * W1.shape[2]
    w1f = np.ascontiguousarray(
        W1.transpose(1, 0, 2).reshape(f, d).astype(np.float16)
    )
    w2f = np.ascontiguousarray(
        W2.transpose(1, 0, 2).reshape(d, d).astype(np.float16)
    )
    woutf = np.ascontiguousarray(Wout.astype(np.float32))
    bbf = np.ascontiguousarray(
        np.broadcast_to(bout.astype(np.float32), (128, Wout.shape[1]))
    )
    adj16 = adj.astype(np.float16)
    x16 = x.astype(np.float16)
    in_maps = []
    for c in range(ncores):
        rows = slice(c * s, (c + 1) * s)
        # rotate contraction rows so this core's own nodes come first
        rot = np.roll(np.arange(n_total), -c * s)
        adjtc = np.ascontiguousarray(adj16[rows][:, rot].T)
        # xc[g*128 + p, ((j*2 + a)*128) + m] = xrot.T[a*128 + p, (g*8 + j)*128 + m]
        xtc = x16[rot].T  # [F, n_total]
        xcf = np.ascontiguousarray(
            xtc.reshape(2, 128, kb8, 8, 128)
            .transpose(2, 1, 3, 0, 4)
            .reshape(kb8 * 128, 8 * f)
        )
        hoffc = np.zeros((1, 8), np.uint32)
        for g in range(ncores - 1):
            hoffc[0, g] = ((c + 1 + g) % ncores) * 128
        in_maps.append(
            {
                "adjt": adjtc,
                "xc": xcf,
                "w1": w1f,
                "w2": w2f,
                "wout": woutf,
                "bb": bbf,
                "hoff": hoffc,
            }
        )
    return in_maps


def kernel(x, adj, W1, W2, Wout, bout):
    from concourse import bass_utils

    x = np.asarray(x)
    adj = np.asarray(adj)
    in_maps = make_in_maps(x, adj, np.asarray(W1), np.asarray(W2),
                           np.asarray(Wout), np.asarray(bout))
    if "nc" not in _nc_cache:
        _nc_cache["nc"] = build_gat_nc()
    res = bass_utils.run_bass_kernel_spmd(
        _nc_cache["nc"], in_maps, core_ids=list(range(NCORES))
    )
    return np.concatenate([r["out"] for r in res.results], axis=0).astype(np.float32)
